# revision 1
# baseline (speedup 1.0000x reference)
"""MultiHeadAttention Trainium2 kernel.

B=4, S=2048, D_IN=D_MODEL=1024, H=16, D_HEAD=64, fp32 in/out.

Sharding over 8 cores: core c handles batch b=c//2 and heads
[(c%2)*8, (c%2)*8+8) (a 512-wide slice of D_MODEL). Each core computes
its heads' attention and a partial output projection; the host sums the
two partials per batch and adds the output bias.

Per-core schedule (all matmuls in float32r = 1 cycle/row):
  X1T/X2T  [D_IN part, S]  transposed on the host, loaded directly
  QT, KT   [512 part (4 chunks of 128), S]   d-on-partitions
  V        [S part (16 chunks of 128), 512]  natural, +ones column (65/head)
  order: QT quarter 0 -> KT,V (all of X2) -> attention per q-block with the
  remaining QT quarters interleaved at q-block boundaries -> output proj.
  scoresT  [S_k part chunk 128, 2 heads, 512 q] psum pair tile
  exp via ACT (scale=1/8 folded in, no max subtraction: |scores/8| < ~3)
  out'     [65, 512] psum accum over k; row 64 = softmax denominator
  normalize: DRAM-bounce broadcast of the denominator, DVE recip+mul
  -> attn_outT [512 part, S]; final = attn_outT.T-slices @ Wo.
"""

import sys

sys.path.insert(0, "/opt/trn_rl_repo")

from contextlib import ExitStack

import numpy as np
import concourse.bass as bass
import concourse.tile as tile
from concourse import bacc, mybir
from concourse.bass_utils import run_bass_kernel_spmd

B, S, D_IN, D_MODEL, H = 4, 2048, 1024, 1024, 16
DH = 64
HPC = 8  # heads per core
DS = 512  # D_MODEL slice per core
F32 = mybir.dt.float32
F32R = mybir.dt.float32r
Exp = mybir.ActivationFunctionType.Exp

NKC = D_IN // 128  # 8  k-chunks for projections
NSC = S // 128  # 16 s-chunks
NDC = DS // 128  # 4  d-chunks of the 512 slice
QB = 512  # attention q-block (matches the s-quarters)
SCALE = 1.0 / np.sqrt(DH)


def _load_xt_quarter(nc, px, x_dram, sq):
    """Load one 512-column quarter of a host-transposed x into SBUF."""
    xt = px.tile([128, NKC, QB], F32R, tag="xt", name="xt")
    for dc in range(NKC):
        nc.sync.dma_start(
            out=xt[:, dc, :],
            in_=x_dram[dc * 128 : (dc + 1) * 128, sq * QB : (sq + 1) * QB],
        )
    return xt


def _emit_qt_quarter(nc, psum_pool, psum_tag, xt, wq_sb, bq_sb, qt, sq, m_range=None):
    """QT[:, :, sq*QB:(sq+1)*QB] = Wq.T-slices @ XT-quarter (+bias)."""
    for m in m_range if m_range is not None else range(NDC):
        ps = psum_pool.tile([128, QB], F32, tag=psum_tag, name="qtp")
        for kc in range(NKC):
            nc.tensor.matmul(
                ps,
                wq_sb[:, kc, m * 128 : (m + 1) * 128],
                xt[:, kc, :],
                start=(kc == 0),
                stop=(kc == NKC - 1),
            )
        nc.vector.tensor_scalar_add(
            qt[:, m, sq * QB : (sq + 1) * QB], ps, bq_sb[:, m : m + 1]
        )


def _kernel_body(nc, tc, aps, phases="ABCD"):
    x1, x2, wq, wk, wv, wo, bq, bk, bv, out = aps

    with ExitStack() as ctx:
        _p = ctx.enter_context(tc.tile_pool(name="pers", bufs=1))

        ones_dram = nc.inline_tensor(
            np.ones((128, 128), np.float32), name="ones_const"
        )

        qt = _p.tile([128, NDC, S], F32R)
        kt = _p.tile([128, NDC, S], F32R)
        v_sb = _p.tile([128, NSC, HPC, DH + 1], F32R)
        bq_sb = _p.tile([128, NDC], F32)
        bk_sb = _p.tile([128, NDC], F32)
        bv_bc = _p.tile([128, DS], F32)

        if "A" not in phases:
            return

        pwq_ctx = ctx.enter_context(ExitStack())
        pwq = pwq_ctx.enter_context(tc.tile_pool(name="pwq", bufs=1))
        wq_sb = pwq.tile([128, NKC, DS], F32R, tag="wq")

        # ---- Phase A: QT quarter 0, then X2 -> KT, V ----
        with ExitStack() as actx:
            px = actx.enter_context(tc.tile_pool(name="phx", bufs=2))
            psp = actx.enter_context(
                tc.tile_pool(name="phpsp", bufs=4, space="PSUM")
            )
            pwkv = actx.enter_context(tc.tile_pool(name="pwkv", bufs=1))
            wk_sb = pwkv.tile([128, NKC, DS], F32R, tag="w0")
            wv_sb = pwkv.tile([128, NKC, DS], F32R, tag="w1")

            # bias loads first (gpsimd queue; they don't contend with the
            # x/w loads on the sync queue) -- the qt-q0 copyback reads bq_sb
            nc.gpsimd.dma_start(
                out=bq_sb, in_=bq.rearrange("(c p) o -> p (c o)", p=128)
            )
            nc.gpsimd.dma_start(
                out=bk_sb, in_=bk.rearrange("(c p) o -> p (c o)", p=128)
            )
            nc.gpsimd.dma_start(
                out=bv_bc,
                in_=bv.rearrange("s o -> o s").to_broadcast([128, DS]),
            )
            # qt quarter 0 first so attention can start right after phase A.
            # interleave the xt and wq chunk loads: the sync HWDGE queue is a
            # FIFO, and matmul kc needs chunk kc of both streams
            xtq0 = px.tile([128, NKC, QB], F32R, tag="xt", name="xtq0")
            for kc in range(NKC):
                nc.sync.dma_start(
                    out=xtq0[:, kc, :], in_=x1[kc * 128 : (kc + 1) * 128, 0:QB]
                )
                nc.sync.dma_start(
                    out=wq_sb[:, kc, :], in_=wq[kc * 128 : (kc + 1) * 128, :]
                )
            _emit_qt_quarter(nc, psp, "pp", xtq0, wq_sb, bq_sb, qt, 0)

            for sq in range(4):
                if sq == 0:
                    xt = px.tile([128, NKC, QB], F32R, tag="xt", name="xtk0")
                    for kc in range(NKC):
                        nc.sync.dma_start(
                            out=xt[:, kc, :],
                            in_=x2[kc * 128 : (kc + 1) * 128, 0:QB],
                        )
                        nc.sync.dma_start(
                            out=wk_sb[:, kc, :],
                            in_=wk[kc * 128 : (kc + 1) * 128, :],
                        )
                        nc.sync.dma_start(
                            out=wv_sb[:, kc, :],
                            in_=wv[kc * 128 : (kc + 1) * 128, :],
                        )
                else:
                    xt = _load_xt_quarter(nc, px, x2, sq)
                # KT quarter
                for m in range(NDC):
                    ps = psp.tile([128, QB], F32, tag="pp", name="ktp")
                    for kc in range(NKC):
                        nc.tensor.matmul(
                            ps,
                            wk_sb[:, kc, m * 128 : (m + 1) * 128],
                            xt[:, kc, :],
                            start=(kc == 0),
                            stop=(kc == NKC - 1),
                        )
                    nc.vector.tensor_scalar_add(
                        kt[:, m, sq * QB : (sq + 1) * QB],
                        ps,
                        bk_sb[:, m : m + 1],
                    )
                # V quarter (natural layout, bias along free dim)
                for sc in range(4):
                    ps = psp.tile([128, QB], F32, tag="pp", name="vp")
                    for kc in range(NKC):
                        nc.tensor.matmul(
                            ps,
                            xt[:, kc, sc * 128 : (sc + 1) * 128],
                            wv_sb[:, kc, :],
                            start=(kc == 0),
                            stop=(kc == NKC - 1),
                        )
                    nc.vector.tensor_add(
                        v_sb[:, sq * 4 + sc, :, 0:DH],
                        ps.rearrange("p (h d) -> p h d", h=HPC),
                        bv_bc.rearrange("p (h d) -> p h d", h=HPC),
                    )

            nc.sync.dma_start(
                out=v_sb[:, :, :, DH : DH + 1],
                in_=ones_dram[:, 0:128]
                .rearrange("p (a b o) -> p a b o", a=NSC, b=HPC)
                .bitcast(F32R),
            )

        if "B" not in phases or "C" not in phases:
            return

        # ---- Phase B+C: attention with interleaved QT quarters ----
        cdctx = ctx.enter_context(ExitStack())
        paot = cdctx.enter_context(tc.tile_pool(name="paot", bufs=1))
        aot = paot.tile([128, NDC, S], F32R)  # attn output, d-on-partitions

        with ExitStack() as bctx:
            pxb = bctx.enter_context(tc.tile_pool(name="pxb", bufs=1))
            psc = bctx.enter_context(tc.tile_pool(name="psc", bufs=2, space="PSUM"))
            pso = bctx.enter_context(tc.tile_pool(name="pso", bufs=4, space="PSUM"))
            pex = bctx.enter_context(tc.tile_pool(name="pex", bufs=6))
            pnr = bctx.enter_context(tc.tile_pool(name="pnr", bufs=1))
            pout2 = bctx.enter_context(tc.tile_pool(name="pout2", bufs=2))
            pdr = bctx.enter_context(tc.tile_pool(name="pdr", bufs=2, space="DRAM"))

            wo_holder = []
            for qb in range(S // QB):
                q0 = qb * QB
                xtn = None
                if qb < 3:
                    # next x1 quarter starts loading behind this q-block
                    xtn = _load_xt_quarter(nc, pxb, x1, qb + 1)
                for pair in range(4):
                    hc = pair  # heads 2*pair (parts 0:64), 2*pair+1 (64:128)
                    outp0 = pso.tile([DH + 1, QB], F32, tag="acc")
                    outp1 = pso.tile([DH + 1, QB], F32, tag="acc")
                    outp = [outp0, outp1]
                    for kc in range(NSC):
                        # both heads' scoresT for this k-chunk in one psum
                        # tile; the two matmuls use disjoint 64-row groups of
                        # the PE array and run concurrently
                        sc_pair = psc.tile([128, 2, QB], F32, tag="sc")
                        for par in range(2):
                            po = par * 64
                            nc.tensor.matmul(
                                sc_pair[:, par, :],
                                kt[po : po + 64, hc, kc * 128 : (kc + 1) * 128],
                                qt[po : po + 64, hc, q0 : q0 + QB],
                                start=True,
                                stop=True,
                            )
                        # one exp instruction covers both heads
                        ex_pair = pex.tile([128, 2, QB], F32R, tag="ex")
                        nc.scalar.activation(
                            ex_pair, sc_pair, Exp, scale=float(SCALE)
                        )
                        for par in range(2):
                            h = 2 * pair + par
                            nc.tensor.matmul(
                                outp[par],
                                v_sb[:, kc, h, :],
                                ex_pair[:, par, :],
                                start=(kc == 0),
                                stop=(kc == NSC - 1),
                            )
                    # normalize both heads of the pair
                    for par in range(2):
                        den = pnr.tile([DH + 1, QB], F32, tag="den")
                        nc.vector.tensor_copy(
                            den[DH : DH + 1, :], outp[par][DH : DH + 1, :]
                        )
                        dscr = pdr.tile([1, QB], F32, tag="dscr")
                        nc.sync.dma_start(out=dscr, in_=den[DH : DH + 1, :])
                        db = pnr.tile([64, QB], F32, tag="db")
                        nc.gpsimd.dma_start(
                            out=db, in_=dscr.to_broadcast([64, QB])
                        )
                        rb = pnr.tile([64, QB], F32, tag="rb")
                        nc.vector.reciprocal(rb, db)
                        if par == 0:
                            nc.vector.tensor_mul(
                                aot[0:64, hc, q0 : q0 + QB],
                                outp[par][0:64, :],
                                rb,
                            )
                        else:
                            tmp = pnr.tile([64, QB], F32R, tag="tmp")
                            nc.vector.tensor_mul(tmp, outp[par][0:64, :], rb)
                            nc.sync.dma_start(
                                out=aot[64:128, hc, q0 : q0 + QB], in_=tmp
                            )
                    if xtn is not None:
                        _emit_qt_quarter(
                            nc, pso, "acc", xtn, wq_sb, bq_sb, qt, qb + 1,
                            m_range=range(pair, pair + 1),
                        )
                    if pair == 3 and qb == 2 and not wo_holder:
                        # prefetch Wo into the wq slot (same size, tag-share;
                        # emitted after the final wq read just above)
                        wo_sb = pwq.tile(
                            [128, NDC, D_MODEL], F32R, tag="wq", name="wo_sb"
                        )
                        for kc in range(NDC):
                            nc.sync.dma_start(
                                out=wo_sb[:, kc, :],
                                in_=wo[kc * 128 : (kc + 1) * 128, :],
                            )
                        wo_holder.append(wo_sb)
                    if (
                        "D" in phases
                        and qb == 3
                        and pair in (0, 1, 2)
                        and wo_holder
                    ):
                        # interleave most of the output projection into the
                        # last q-block's ACT-bound stretch (aot rows for q <
                        # 1536 are complete after qb 2)
                        for mb in range(2 * pair, 2 * pair + 2):
                            ot = pout2.tile(
                                [128, D_MODEL], F32, tag="ot2", name="oti"
                            )
                            for nt in range(2):
                                ps = pso.tile(
                                    [128, 512], F32, tag="acc", name="pfi"
                                )
                                for kc in range(NDC):
                                    nc.tensor.matmul(
                                        ps,
                                        aot[:, kc, mb * 128 : (mb + 1) * 128],
                                        wo_holder[0][
                                            :, kc, nt * 512 : (nt + 1) * 512
                                        ],
                                        start=(kc == 0),
                                        stop=(kc == NDC - 1),
                                    )
                                nc.vector.tensor_copy(
                                    ot[:, nt * 512 : (nt + 1) * 512], ps
                                )
                            eng = nc.sync if mb % 2 == 0 else nc.scalar
                            eng.dma_start(
                                out=out[mb * 128 : (mb + 1) * 128, :], in_=ot
                            )


        # ---- Phase D: output projection ----
        if "D" not in phases:
            return
        with ExitStack() as dctx:
            wo_sb = wo_holder[0]
            psf = dctx.enter_context(tc.tile_pool(name="psf", bufs=4, space="PSUM"))
            pout = dctx.enter_context(tc.tile_pool(name="pout", bufs=4))
            for mb in range(6, NSC):
                ot = pout.tile([128, D_MODEL], F32, tag="ot")
                for nt in range(2):
                    ps = psf.tile([128, 512], F32, tag="pf")
                    for kc in range(NDC):
                        nc.tensor.matmul(
                            ps,
                            aot[:, kc, mb * 128 : (mb + 1) * 128],
                            wo_sb[:, kc, nt * 512 : (nt + 1) * 512],
                            start=(kc == 0),
                            stop=(kc == NDC - 1),
                        )
                    nc.vector.tensor_copy(ot[:, nt * 512 : (nt + 1) * 512], ps)
                eng = nc.sync if mb % 2 == 0 else nc.scalar
                eng.dma_start(out=out[mb * 128 : (mb + 1) * 128, :], in_=ot)


_NC_CACHE = []


def _build(phases="ABCD"):
    if phases == "ABCD" and _NC_CACHE:
        return _NC_CACHE[0]
    nc = bacc.Bacc(None, target_bir_lowering=False, debug=False)
    x1 = nc.dram_tensor("x1", [D_IN, S], F32R, kind="ExternalInput")
    x2 = nc.dram_tensor("x2", [D_IN, S], F32R, kind="ExternalInput")
    wq = nc.dram_tensor("wq", [D_IN, DS], F32R, kind="ExternalInput")
    wk = nc.dram_tensor("wk", [D_IN, DS], F32R, kind="ExternalInput")
    wv = nc.dram_tensor("wv", [D_IN, DS], F32R, kind="ExternalInput")
    wo = nc.dram_tensor("wo", [DS, D_MODEL], F32R, kind="ExternalInput")
    bq = nc.dram_tensor("bq", [DS, 1], F32, kind="ExternalInput")
    bk = nc.dram_tensor("bk", [DS, 1], F32, kind="ExternalInput")
    bv = nc.dram_tensor("bv", [DS, 1], F32, kind="ExternalInput")
    out = nc.dram_tensor("out", [S, D_MODEL], F32, kind="ExternalOutput")
    with tile.TileContext(nc) as tc:
        _kernel_body(
            nc,
            tc,
            phases=phases,
            aps=(
                x1[:, :],
                x2[:, :],
                wq[:, :],
                wk[:, :],
                wv[:, :],
                wo[:, :],
                bq[:, :],
                bk[:, :],
                bv[:, :],
                out[:, :],
            ),
        )
    nc.compile()
    if phases == "ABCD":
        _NC_CACHE.append(nc)
    return nc


def _run(inputs, trace=False, **kw):
    nc = _build()
    f32 = lambda a: np.ascontiguousarray(np.asarray(a, dtype=np.float32))
    X1, X2 = f32(inputs["X1"]), f32(inputs["X2"])
    in_maps = []
    for c in range(8):
        b, hf = c // 2, c % 2
        sl = slice(hf * DS, (hf + 1) * DS)
        in_maps.append(
            {
                "x1": np.ascontiguousarray(X1[b].T),
                "x2": np.ascontiguousarray(X2[b].T),
                "wq": f32(inputs["Wq"])[:, sl],
                "wk": f32(inputs["Wk"])[:, sl],
                "wv": f32(inputs["Wv"])[:, sl],
                "wo": f32(inputs["Wo"])[sl, :],
                "bq": f32(inputs["bq"])[sl].reshape(DS, 1),
                "bk": f32(inputs["bk"])[sl].reshape(DS, 1),
                "bv": f32(inputs["bv"])[sl].reshape(DS, 1),
            }
        )
    res = run_bass_kernel_spmd(nc, in_maps, list(range(8)), trace=trace, **kw)
    parts = [res.results[c]["out"] for c in range(8)]
    bo = f32(inputs["bo"])
    full = np.stack(
        [parts[2 * b] + parts[2 * b + 1] + bo[None, :] for b in range(B)]
    )
    return full.astype(np.float32), res


def kernel(**inputs):
    out, _ = _run(inputs, trace=False)
    return out

